# revision 42
# baseline (speedup 1.0000x reference)
"""nn_GBEncoderBlock on 8 TRN2 NeuronCores.

Sharding: data-parallel over batch — 1 batch element per core, SPMD, no
collectives.  Per-core layouts:
  - residual stream x: SBUF [128(l%128), 8(l//128), 512(h)] f32
  - conv/matmul operand layouts [H,L] obtained via XBAR DMA-transpose (bf16)
  - LayerNorms computed natively in [L,H] via bn_stats/bn_aggr (f32 stats)
  - depthwise conv k=7: PE matmuls with diagonal weight blocks accumulating
    the 7 taps in PSUM (LN gamma folded into the diagonals, beta+bias into
    the ACT epilogue)
  - scores computed transposed [m,l] so the key mask enters as ACT's
    per-partition bias in a fused exp(s/8 + bias); DK=64 head pairs packed
    into the PE via tile_position row strips
  - softmax rowsums via a ones-column appended to V; flash-style
    normalization after AV
All matmuls bf16 (tolerance 2e-2), psum f32, 512-wide psum tiles (1 bank).
"""

import numpy as np
import ml_dtypes

import concourse.bass as bass
from concourse import bacc
import concourse.mybir as mybir
import concourse.tile as tile

B, L, H = 8, 1024, 512
NHEAD, DK = 8, 64
KSZ, NLAYERS = 7, 4
FFN = 4 * H
EPS = 1e-6
P = 128
LT = L // P    # 8 l-tiles
HS = H // P    # 4 h-subtiles
FS = FFN // P  # 16
PAD = KSZ // 2
MASK_NEG = -30.0
LH = 512       # psum free width (one bank)
NLH = L // LH  # 2 l-halves

f32 = mybir.dt.float32
bf16 = mybir.dt.bfloat16
u8 = mybir.dt.uint8
FT = mybir.ActivationFunctionType
OP = mybir.AluOpType

# debug: "conv", "attn", "all" — where to stop emitting (sim bisection)
PHASES = "all"


def _bcast_ap(row_ap):
    """[N] DRAM AP -> [P, N] AP replicated across partitions (step-0)."""
    return bass.AP(
        tensor=row_ap.tensor,
        offset=row_ap.offset,
        ap=[[0, P]] + [list(d) for d in row_ap.ap],
    )


def _pp(vec_ap, s):
    """[s*P] DRAM AP -> [P, s] per-partition layout (h = s_idx*P + p)."""
    return vec_ap.rearrange("(s p) -> p s", p=P)


def build_nc():
    nc = bacc.Bacc()

    d = {}
    d["x_d"] = nc.dram_tensor("x", [L, H], f32, kind="ExternalInput")
    d["mask_d"] = nc.dram_tensor("mask", [L], u8, kind="ExternalInput")
    d["pos_d"] = nc.dram_tensor("pos", [L, H], f32, kind="ExternalInput")
    d["cg_d"] = nc.dram_tensor("cg", [NLAYERS, H], f32, kind="ExternalInput")
    d["cb_d"] = nc.dram_tensor("cb", [NLAYERS, H], f32, kind="ExternalInput")
    d["dww_d"] = nc.dram_tensor("dww", [NLAYERS, H, KSZ], f32, kind="ExternalInput")
    d["dwb_d"] = nc.dram_tensor("dwb", [NLAYERS, H], f32, kind="ExternalInput")
    d["pwt_d"] = nc.dram_tensor("pwt", [NLAYERS, H, H], bf16, kind="ExternalInput")
    d["pwb_d"] = nc.dram_tensor("pwb", [NLAYERS, H], f32, kind="ExternalInput")
    d["ag_d"] = nc.dram_tensor("ag", [H], f32, kind="ExternalInput")
    d["ab_d"] = nc.dram_tensor("ab", [H], f32, kind="ExternalInput")
    d["wq_d"] = nc.dram_tensor("wq", [H, H], bf16, kind="ExternalInput")
    d["wk_d"] = nc.dram_tensor("wk", [H, H], bf16, kind="ExternalInput")
    d["wv_d"] = nc.dram_tensor("wv", [H, H], bf16, kind="ExternalInput")
    d["pjt_d"] = nc.dram_tensor("pjt", [H, H], bf16, kind="ExternalInput")
    d["pjb_d"] = nc.dram_tensor("pjb", [H], f32, kind="ExternalInput")
    d["fg_d"] = nc.dram_tensor("fg", [H], f32, kind="ExternalInput")
    d["fb_d"] = nc.dram_tensor("fb", [H], f32, kind="ExternalInput")
    d["w1t_d"] = nc.dram_tensor("w1t", [H, FFN], bf16, kind="ExternalInput")
    d["b1_d"] = nc.dram_tensor("b1", [FFN], f32, kind="ExternalInput")
    d["w2t_d"] = nc.dram_tensor("w2t", [FFN, H], bf16, kind="ExternalInput")
    d["b2_d"] = nc.dram_tensor("b2", [H], f32, kind="ExternalInput")
    d["out_d"] = nc.dram_tensor("out", [L, H], f32, kind="ExternalOutput")

    with tile.TileContext(nc) as tc:
        with (
            tc.tile_pool(name="persist", bufs=1) as pp,
            tc.tile_pool(name="w8", bufs=6) as w8,
            tc.tile_pool(name="p16", bufs=2) as p16,
            tc.tile_pool(name="small", bufs=3) as sm,
            tc.tile_pool(name="psum", bufs=1, space="PSUM") as psp,
        ):
            d.update(pp=pp, w8=w8, p16=p16, sm=sm, psp=psp)
            emit(nc, d)
    nc.finalize()
    return nc


def emit(nc, env):
    pp, w8, p16, sm, psp = (
        env["pp"], env["w8"], env["p16"], env["sm"], env["psp"])
    x_d, mask_d, pos_d = env["x_d"], env["mask_d"], env["pos_d"]
    cg_d, cb_d = env["cg_d"], env["cb_d"]
    dww_d, dwb_d, pwt_d, pwb_d = env["dww_d"], env["dwb_d"], env["pwt_d"], env["pwb_d"]
    ag_d, ab_d = env["ag_d"], env["ab_d"]
    wq_d, wk_d, wv_d = env["wq_d"], env["wk_d"], env["wv_d"]
    pjt_d, pjb_d = env["pjt_d"], env["pjb_d"]
    fg_d, fb_d = env["fg_d"], env["fb_d"]
    w1t_d, b1_d, w2t_d, b2_d = env["w1t_d"], env["b1_d"], env["w2t_d"], env["b2_d"]
    out_d = env["out_d"]

    ts = bass.ts

    def psum_sc(name):
        return psp.tile([P, LH], f32, name=name, tag="sc", bufs=6)

    # ---------------- constants / persistent loads ----------------
    x_sb = pp.tile([P, LT, H], f32, name="x_sb")
    nc.sync.dma_start(out=x_sb, in_=x_d[:, :].rearrange("(lo p) h -> p lo h", p=P))

    pos_r = pos_d[:, :].rearrange("(lo p) h -> p lo h", p=P)

    mask_u = pp.tile([P, LT], u8, name="mask_u")
    nc.gpsimd.dma_start(out=mask_u, in_=mask_d[:].rearrange("(mo p) -> p mo", p=P))
    maskb = pp.tile([P, LT], f32, name="maskb")
    nc.vector.tensor_scalar_mul(maskb, mask_u, MASK_NEG)

    dww_sb = pp.tile([P, NLAYERS, HS, KSZ], f32, name="dww_sb")
    dwb_sb = pp.tile([P, NLAYERS, HS], f32, name="dwb_sb")
    pwb_sb = pp.tile([P, NLAYERS, HS], f32, name="pwb_sb")
    for i in range(NLAYERS):
        nc.sync.dma_start(
            out=dww_sb[:, i], in_=dww_d[i, :, :].rearrange("(s p) k -> p s k", p=P))
        nc.gpsimd.dma_start(out=dwb_sb[:, i], in_=_pp(dwb_d[i, :], HS))
        nc.gpsimd.dma_start(out=pwb_sb[:, i], in_=_pp(pwb_d[i, :], HS))
    pjb_sb = pp.tile([P, HS], f32, name="pjb_sb")
    nc.gpsimd.dma_start(out=pjb_sb, in_=_pp(pjb_d[:], HS))
    b1_sb = pp.tile([P, FS], f32, name="b1_sb")
    nc.gpsimd.dma_start(out=b1_sb, in_=_pp(b1_d[:], FS))
    b2_sb = pp.tile([P, HS], f32, name="b2_sb")
    nc.gpsimd.dma_start(out=b2_sb, in_=_pp(b2_d[:], HS))

    # x += pos via accumulate-DMA
    nc.gpsimd.dma_start(out=x_sb, in_=pos_r, accum_op=OP.add)

    # LN gamma/beta in per-partition [H,*] layout: conv pairs fold into the
    # depthwise diagonals; attn/ffn pairs apply on the transposed z tiles
    gbx = pp.tile([P, 2, 2, HS], f32, name="gbx")
    for j, row in enumerate((ag_d[:], ab_d[:], fg_d[:], fb_d[:])):
        nc.gpsimd.dma_start(out=gbx[:, j // 2, j % 2], in_=_pp(row, HS))

    # conv-LN gamma/beta (folded into depthwise)
    gpp_sb = pp.tile([P, NLAYERS, HS], f32, name="gpp_sb")
    bpp_sb = pp.tile([P, NLAYERS, HS], f32, name="bpp_sb")
    for i in range(NLAYERS):
        nc.gpsimd.dma_start(out=gpp_sb[:, i], in_=_pp(cg_d[i, :], HS))
        nc.gpsimd.dma_start(out=bpp_sb[:, i], in_=_pp(cb_d[i, :], HS))

    from concourse.masks import make_identity
    ident = pp.tile([P, P], bf16, name="ident")
    make_identity(nc, ident)

    def layer_norm(nm):
        """LN over H of x_sb -> z_lh [P, LT, H] bf16, gamma/beta applied
        downstream in the transposed layout (torch-style unbiased std)."""
        stats = sm.tile([P, LT, 6], f32, name="st" + nm, tag="stats")
        mv = sm.tile([P, LT, 2], f32, name="mv" + nm, tag="mv")
        for lo in range(LT):
            nc.vector.bn_stats(out=stats[:, lo], in_=x_sb[:, lo])
            nc.vector.bn_aggr(out=mv[:, lo], in_=stats[:, lo])
        std = sm.tile([P, LT], f32, name="sd" + nm, tag="std")
        nc.scalar.activation(out=std, in_=mv[:, :, 1], func=FT.Sqrt,
                             scale=float(H) / (H - 1))
        nc.vector.tensor_scalar_add(std, std, EPS)
        rstd = sm.tile([P, LT], f32, name="rs" + nm, tag="rstd")
        nc.vector.reciprocal(out=rstd, in_=std)
        bm = sm.tile([P, LT], f32, name="bm" + nm, tag="bm")
        nc.vector.tensor_mul(bm, mv[:, :, 0], rstd)
        nc.vector.tensor_scalar_mul(bm, bm, -1.0)
        z_lh = w8.tile([P, LT, H], bf16, name="zlh" + nm, tag="w8")
        for lo in range(LT):
            nc.scalar.activation(
                out=z_lh[:, lo], in_=x_sb[:, lo], func=FT.Identity,
                scale=rstd[:, lo:lo + 1], bias=bm[:, lo:lo + 1])
        return z_lh

    def transpose_lh_to_int(z_lh, nm, gb_j):
        """[P,LT,H] -> one-shot XBAR transpose -> [P(h%128), LT, HS, P(l%128)],
        then gamma/beta (per-partition here) per h-subtile."""
        zt = w8.tile([P, LT, HS, P], bf16, name="zint" + nm, tag="w8")
        nc.sync.dma_start_transpose(zt, z_lh[:, :, :])
        for ks in range(HS):
            nc.vector.tensor_scalar(
                out=zt[:, :, ks, :], in0=zt[:, :, ks, :],
                scalar1=gbx[:, gb_j, 0, ks:ks + 1],
                scalar2=gbx[:, gb_j, 1, ks:ks + 1], op0=OP.mult, op1=OP.add)
        return zt

    # ---------------- conv layers ----------------
    for i in range(NLAYERS):
        # depthwise as PE diag-matmuls; LN gamma folds into the diagonals,
        # (LN beta + dw bias) folds into the psum epilogue bias:
        #   dw(g*z+b) = sum_k w_k*g (.) shift_k(z) + (dwb + b*sum_k w_k)
        wg = sm.tile([P, HS, KSZ], f32, name=f"wg{i}", tag="wg", bufs=2)
        nc.vector.tensor_mul(
            wg, dww_sb[:, i],
            gpp_sb[:, i].unsqueeze(-1).to_broadcast([P, HS, KSZ]))
        dmat = w8.tile([P, HS, KSZ, P], bf16, name=f"dmat{i}", tag="w8")
        nc.vector.tensor_mul(
            dmat,
            ident.unsqueeze(1).unsqueeze(1).to_broadcast([P, HS, KSZ, P]),
            wg.unsqueeze(-1).to_broadcast([P, HS, KSZ, P]))
        swk = sm.tile([P, HS], f32, name=f"swk{i}", tag="swk", bufs=2)
        nc.vector.tensor_reduce(out=swk, in_=dww_sb[:, i],
                                axis=mybir.AxisListType.X, op=OP.add)
        bprime = sm.tile([P, HS], f32, name=f"bp{i}", tag="bprime", bufs=2)
        nc.vector.tensor_mul(bprime, swk, bpp_sb[:, i])
        nc.vector.tensor_add(bprime, bprime, dwb_sb[:, i])

        # LN stats
        stats = sm.tile([P, LT, 6], f32, name=f"stc{i}", tag="stats")
        mv = sm.tile([P, LT, 2], f32, name=f"mvc{i}", tag="mv")
        for lo in range(LT):
            nc.vector.bn_stats(out=stats[:, lo], in_=x_sb[:, lo])
            nc.vector.bn_aggr(out=mv[:, lo], in_=stats[:, lo])
        std = sm.tile([P, LT], f32, name=f"sdc{i}", tag="std")
        nc.scalar.activation(out=std, in_=mv[:, :, 1], func=FT.Sqrt,
                             scale=float(H) / (H - 1))
        nc.vector.tensor_scalar_add(std, std, EPS)
        rstd = sm.tile([P, LT], f32, name=f"rsc{i}", tag="rstd")
        nc.vector.reciprocal(out=rstd, in_=std)
        bm = sm.tile([P, LT], f32, name=f"bmc{i}", tag="bm")
        nc.vector.tensor_mul(bm, mv[:, :, 0], rstd)
        nc.vector.tensor_scalar_mul(bm, bm, -1.0)

        pwt_sb = w8.tile([P, HS, H], bf16, name=f"pwt{i}", tag="w8")
        nc.sync.dma_start(
            out=pwt_sb, in_=pwt_d[i, :, :].rearrange("(s p) o -> p s o", p=P))

        z_lh = w8.tile([P, LT, H], bf16, name=f"zlhc{i}", tag="w8")
        # per-l-half padded [H,L] tiles so dw(lh) starts as soon as its half
        # (plus the 3-col halo) is transposed
        zh = [w8.tile([P, HS, LH + 2 * PAD], bf16, name=f"zhl{i}_{j}",
                      tag="w8p", bufs=4) for j in range(NLH)]
        nc.vector.memset(zh[0][:, :, 0:PAD], 0.0)
        nc.vector.memset(zh[1][:, :, LH + PAD:LH + 2 * PAD], 0.0)
        ach = [sm.tile([P, HS, LH], bf16, name=f"acc{i}_{j}", tag="acch",
                       bufs=5) for j in range(NLH)]
        hh = [sm.tile([P, HS, LH], bf16, name=f"hhl{i}_{j}", tag="acch",
                      bufs=5) for j in range(NLH)]

        def conv_apply_transpose(lo):
            nc.scalar.activation(
                out=z_lh[:, lo], in_=x_sb[:, lo], func=FT.Identity,
                scale=rstd[:, lo:lo + 1], bias=bm[:, lo:lo + 1])
            ztmp = sm.tile([P, HS, P], bf16, name=f"ztmp{i}_{lo}", tag="ztmp",
                           bufs=4)
            nc.sync.dma_start_transpose(ztmp, z_lh[:, lo, :])
            j, c = lo // 4, (lo % 4) * P
            nc.vector.tensor_copy(out=zh[j][:, :, PAD + c:PAD + c + P],
                                  in_=ztmp)
            if lo == 4:  # halo: first PAD cols of lh1 -> right edge of lh0
                nc.vector.tensor_copy(out=zh[0][:, :, PAD + LH:],
                                      in_=ztmp[:, :, 0:PAD])
            if lo == 3:  # halo: last PAD cols of lh0 -> left edge of lh1
                nc.vector.tensor_copy(out=zh[1][:, :, 0:PAD],
                                      in_=ztmp[:, :, P - PAD:P])

        def conv_dw(lh):
            # depthwise on PE: psum[c',l] += D_k[c,c']z[c,l+k], D_k=diag(w_k*g)
            # split into two 64x64 diagonal quadrants packed at (0,0)/(64,64)
            for s in range(HS):
                ps = psum_sc(f"dwps{i}_{s}_{lh}")
                for k in range(KSZ):
                    nc.tensor.matmul(
                        ps, dmat[:, s, k, :], zh[lh][:, s, k:k + LH],
                        start=(k == 0), stop=(k == KSZ - 1))
                nc.scalar.activation(
                    out=ach[lh][:, s], in_=ps,
                    func=FT.Identity, bias=bprime[:, s:s + 1], scale=1.0)

        def conv_pw(lh):
            for ot in range(HS):
                ps = psum_sc(f"pwps{i}_{ot}_{lh}")
                for ks in range(HS):
                    nc.tensor.matmul(
                        ps, pwt_sb[:, ks, ts(ot, P)], ach[lh][:, ks],
                        start=(ks == 0), stop=(ks == HS - 1))
                nc.scalar.activation(
                    out=hh[lh][:, ot], in_=ps,
                    func=FT.Relu, bias=pwb_sb[:, i, ot:ot + 1], scale=1.0)

        def conv_tail(lh):
            ht = sm.tile([P, HS, 4, P], bf16, name=f"ht{i}_{lh}", tag="acch",
                         bufs=5)
            nc.sync.dma_start_transpose(ht, hh[lh][:, :, :])
            for lo in range(4 * lh, 4 * lh + 4):
                xv = x_sb[:, lo].rearrange("p (a b) -> p a b", b=P)
                eng = nc.gpsimd if lo % 3 == 2 else nc.vector
                eng.tensor_add(xv, xv, ht[:, :, lo % 4, :])

        # pipeline the two l-halves
        for lo in range(5):
            conv_apply_transpose(lo)
        conv_dw(0)
        for lo in range(5, LT):
            conv_apply_transpose(lo)
        conv_pw(0)
        conv_dw(1)
        conv_tail(0)
        conv_pw(1)
        conv_tail(1)

    if PHASES == "conv":
        nc.sync.dma_start(
            out=out_d[:, :].rearrange("(lo p) h -> p lo h", p=P), in_=x_sb)
        return

    # ---------------- attention ----------------
    z_lh = layer_norm("a")
    zq_t = transpose_lh_to_int(z_lh, "a", 0)  # [P(h), LT, HS, P(l)]

    # per-headpair q/k/v tiles so scores(hp) starts as soon as its pair done
    q_t, k_t, v_t = ({}, {}, {})
    vt_r = {}
    for wname, w_d, store in (("q", wq_d, q_t), ("k", wk_d, k_t),
                              ("v", wv_d, v_t)):
        w_sb = w8.tile([P, HS, H], bf16, name=f"w{wname}sb", tag="w8")
        nc.sync.dma_start(out=w_sb, in_=w_d[:, :].rearrange("(s p) o -> p s o", p=P))
        for ot in range(HS):
            prj = sm.tile([P, L], bf16, name=f"{wname}sb{ot}", tag="qkv",
                          bufs=16)
            store[ot] = prj
            for lh in range(NLH):
                ps = psum_sc(f"qkvps{wname}{ot}_{lh}")
                for ks in range(HS):
                    nc.tensor.matmul(
                        ps, w_sb[:, ks, ts(ot, P)],
                        zq_t[:, lh * (LT // NLH):(lh + 1) * (LT // NLH), ks, :],
                        start=(ks == 0), stop=(ks == HS - 1))
                dst = prj[:, lh * LH:(lh + 1) * LH]
                if wname == "v":
                    nc.scalar.copy(out=dst, in_=ps)
                else:
                    nc.vector.tensor_copy(out=dst, in_=ps)
            if wname == "v":
                # V^T per pair: [dv(2 heads), m] -> [m, dv]
                vr = sm.tile([P, LT, P], bf16, name=f"vtr{ot}", tag="qkv",
                             bufs=16)
                nc.sync.dma_start_transpose(vr, prj[:, :])
                vt_r[ot] = vr

    vt2 = w8.tile([P, LT, NHEAD, DK + 1], bf16, name="vt2", tag="w8")
    for hp in range(HS):
        for mo in range(LT):
            nc.vector.tensor_copy(out=vt2[:, mo, 2 * hp, 0:DK],
                                  in_=vt_r[hp][:, mo, 0:DK])
            nc.vector.tensor_copy(out=vt2[:, mo, 2 * hp + 1, 0:DK],
                                  in_=vt_r[hp][:, mo, DK:2 * DK])
    for h in range(NHEAD):
        nc.vector.memset(vt2[:, :, h, DK:DK + 1], 1.0)

    oT_sb = w8.tile([P, HS, L], bf16, name="oT_sb", tag="w8")
    ones1 = pp.tile([P, DK], bf16, name="ones1")
    nc.vector.memset(ones1, 1.0)
    sc_scale = 1.0 / float(np.sqrt(DK))
    for hp in range(HS):
        hA, hB = 2 * hp, 2 * hp + 1
        pA = [p16.tile([P, LT, LH], bf16, name=f"pA{hp}_{j}", tag="p16",
                       bufs=4) for j in range(NLH)]
        pB = [p16.tile([P, LT, LH], bf16, name=f"pB{hp}_{j}", tag="p16",
                       bufs=4) for j in range(NLH)]
        for lh in range(NLH):
            for mo in range(LT):
                psA = psum_sc(f"sA{hp}_{mo}_{lh}")
                psB = psum_sc(f"sB{hp}_{mo}_{lh}")
                nc.tensor.matmul(
                    psA, k_t[hp][0:DK, ts(mo, P)],
                    q_t[hp][0:DK, lh * LH:(lh + 1) * LH],
                    start=True, stop=True, tile_position=(0, 0))
                nc.tensor.matmul(
                    psB, k_t[hp][DK:P, ts(mo, P)],
                    q_t[hp][DK:P, lh * LH:(lh + 1) * LH],
                    start=True, stop=True, tile_position=(DK, 0))
                nc.scalar.activation(
                    out=pA[lh][:, mo, :], in_=psA, func=FT.Exp,
                    bias=maskb[:, mo:mo + 1], scale=sc_scale)
                nc.scalar.activation(
                    out=pB[lh][:, mo, :], in_=psB, func=FT.Exp,
                    bias=maskb[:, mo:mo + 1], scale=sc_scale)

        for (hh, ph, part0) in ((hA, pA, True), (hB, pB, False)):
            rb = sm.tile([P, L], bf16, name=f"rb{hh}", tag="rbc", bufs=2)
            if not part0:
                otmp = sm.tile([DK, L], bf16, name=f"ot{hh}", tag="otmp", bufs=2)
            for lhx in range(NLH):
                lsl = slice(lhx * LH, (lhx + 1) * LH)
                rtmp = sm.tile([P, LH], bf16, name=f"rt{hh}_{lhx}", tag="rtmp",
                               bufs=2)
                r0 = sm.tile([P, LH], bf16, name=f"r0{hh}_{lhx}", tag="r0",
                             bufs=2)
                pso = psp.tile([DK + 1, LH], f32, name=f"av{hh}_{lhx}",
                               tag="av", bufs=2)
                for mo in range(LT):
                    nc.tensor.matmul(pso, vt2[:, mo, hh, 0:DK + 1],
                                     ph[lhx][:, mo, :],
                                     start=(mo == 0), stop=(mo == LT - 1))
                # reciprocal of rowsum (lives at partition DK=64)
                with nc.allow_low_precision(reason="softmax denom in bf16"):
                    nc.vector.reciprocal(out=rtmp[DK:DK + 1, :],
                                         in_=pso[DK:DK + 1, :])
                nc.sync.dma_start(out=r0[0:1, :], in_=rtmp[DK:DK + 1, :])
                # broadcast across DK partitions: ones [1,DK] outer r0 [1,LH]
                psR = psum_sc(f"psR{hh}_{lhx}")
                nc.tensor.matmul(psR[0:DK, :], ones1[0:1, :], r0[0:1, :],
                                 start=True, stop=True)
                nc.scalar.copy(out=rb[0:DK, lsl], in_=psR[0:DK, :])
                if part0:
                    nc.vector.scalar_tensor_tensor(
                        out=oT_sb[0:DK, hp, lsl], in0=pso[0:DK, :], scalar=0.0,
                        in1=rb[0:DK, lsl], op0=OP.bypass, op1=OP.mult)
                else:
                    nc.vector.scalar_tensor_tensor(
                        out=otmp[:, lsl], in0=pso[0:DK, :], scalar=0.0,
                        in1=rb[0:DK, lsl], op0=OP.bypass, op1=OP.mult)
            if not part0:
                nc.sync.dma_start(out=oT_sb[DK:P, hp, :], in_=otmp)

    # output projection + residual (per l-half tails)
    pjt_sb = w8.tile([P, HS, H], bf16, name="pjt_sb", tag="w8")
    nc.sync.dma_start(out=pjt_sb, in_=pjt_d[:, :].rearrange("(s p) o -> p s o", p=P))
    for lh in range(NLH):
        pr_h = sm.tile([P, HS, LH], bf16, name=f"prh{lh}", tag="acch", bufs=5)
        for ot in range(HS):
            ps = psum_sc(f"prps{ot}_{lh}")
            for ds in range(HS):
                nc.tensor.matmul(ps, pjt_sb[:, ds, ts(ot, P)],
                                 oT_sb[:, ds, lh * LH:(lh + 1) * LH],
                                 start=(ds == 0), stop=(ds == HS - 1))
            nc.scalar.activation(
                out=pr_h[:, ot], in_=ps,
                func=FT.Identity, bias=pjb_sb[:, ot:ot + 1], scale=1.0)
        prt = sm.tile([P, HS, 4, P], bf16, name=f"prt{lh}", tag="acch", bufs=5)
        nc.sync.dma_start_transpose(prt, pr_h[:, :, :])
        for lo in range(4 * lh, 4 * lh + 4):
            xv = x_sb[:, lo].rearrange("p (a b) -> p a b", b=P)
            eng = nc.gpsimd if lo % 3 == 2 else nc.vector
            eng.tensor_add(xv, xv, prt[:, :, lo % 4, :])

    if PHASES == "attn":
        nc.sync.dma_start(
            out=out_d[:, :].rearrange("(lo p) h -> p lo h", p=P), in_=x_sb)
        return

    # ---------------- FFN ----------------
    z_lh = layer_norm("f")
    zf_t = transpose_lh_to_int(z_lh, "f", 1)

    # h1 in 4 quarter tiles so W2's accumulation starts after each quarter
    h1q = [w8.tile([P, FS // 4, L], bf16, name=f"h1q{j}", tag="w8")
           for j in range(4)]
    w2_sb = w8.tile([P, FS, H], bf16, name="w2_sb", tag="w2", bufs=1)
    nc.sync.dma_start(out=w2_sb, in_=w2t_d[:, :].rearrange("(s p) o -> p s o", p=P))
    for ot in range(FS):
        wslice = sm.tile([P, HS, P], bf16, name=f"w1s{ot}", tag="wsl", bufs=4)
        nc.sync.dma_start(
            out=wslice,
            in_=w1t_d[:, ts(ot, P)].rearrange("(s p) o -> p s o", p=P))
        htile = h1q[ot // (FS // 4)]
        for lh in range(NLH):
            ps = psum_sc(f"f1ps{ot}_{lh}")
            for ks in range(HS):
                nc.tensor.matmul(
                    ps, wslice[:, ks, :],
                    zf_t[:, lh * (LT // NLH):(lh + 1) * (LT // NLH), ks, :],
                    start=(ks == 0), stop=(ks == HS - 1))
            nc.scalar.activation(
                out=htile[:, ot % (FS // 4), lh * LH:(lh + 1) * LH], in_=ps,
                func=FT.Relu, bias=b1_sb[:, ot:ot + 1], scale=1.0)

    for lh in range(NLH):
        o2_h = sm.tile([P, HS, LH], bf16, name=f"o2h{lh}", tag="acch", bufs=5)
        for ot in range(HS):
            ps = psum_sc(f"f2ps{ot}_{lh}")
            for ks in range(FS):
                htile = h1q[ks // (FS // 4)]
                nc.tensor.matmul(ps, w2_sb[:, ks, ts(ot, P)],
                                 htile[:, ks % (FS // 4), lh * LH:(lh + 1) * LH],
                                 start=(ks == 0), stop=(ks == FS - 1))
            nc.scalar.activation(
                out=o2_h[:, ot], in_=ps,
                func=FT.Identity, bias=b2_sb[:, ot:ot + 1], scale=1.0)
        o2t = sm.tile([P, HS, 4, P], bf16, name=f"o2t{lh}", tag="acch", bufs=5)
        nc.sync.dma_start_transpose(o2t, o2_h[:, :, :])
        for lo in range(4 * lh, 4 * lh + 4):
            xv = x_sb[:, lo].rearrange("p (a b) -> p a b", b=P)
            eng = nc.gpsimd if lo % 3 == 2 else nc.vector
            eng.tensor_add(xv, xv, o2t[:, :, lo % 4, :])

    nc.sync.dma_start(
        out=out_d[:, :].rearrange("(lo p) h -> p lo h", p=P), in_=x_sb)


_NC_CACHE = {}
LAST_RESULTS = None


def get_nc():
    if "nc" not in _NC_CACHE:
        _NC_CACHE["nc"] = build_nc()
    return _NC_CACHE["nc"]


def prep_inputs(x, x_mask, pos_emb, cnn_gamma, cnn_beta, cnn_dw_w, cnn_dw_b,
                cnn_pw_w, cnn_pw_b, attn_gamma, attn_beta, w_qs, w_ks, w_vs,
                proj_w, proj_b, ffn_gamma, ffn_beta, ffn_w1, ffn_b1,
                ffn_w2, ffn_b2):
    bf = ml_dtypes.bfloat16
    f = np.float32
    C = np.ascontiguousarray

    x = np.asarray(x, f)
    mask = np.asarray(x_mask).astype(np.uint8)
    base = dict(
        pos=C(np.asarray(pos_emb, f)[0, :L, :]),
        cg=C(np.asarray(cnn_gamma, f)), cb=C(np.asarray(cnn_beta, f)),
        dww=C(np.asarray(cnn_dw_w, f)), dwb=C(np.asarray(cnn_dw_b, f)),
        pwt=C(np.transpose(np.asarray(cnn_pw_w, f), (0, 2, 1))).astype(bf),
        pwb=C(np.asarray(cnn_pw_b, f)),
        ag=C(np.asarray(attn_gamma, f)), ab=C(np.asarray(attn_beta, f)),
        wq=C(np.transpose(np.asarray(w_qs, f), (1, 0, 2)).reshape(H, H)).astype(bf),
        wk=C(np.transpose(np.asarray(w_ks, f), (1, 0, 2)).reshape(H, H)).astype(bf),
        wv=C(np.transpose(np.asarray(w_vs, f), (1, 0, 2)).reshape(H, H)).astype(bf),
        pjt=C(np.asarray(proj_w, f).T).astype(bf),
        pjb=C(np.asarray(proj_b, f)),
        fg=C(np.asarray(ffn_gamma, f)), fb=C(np.asarray(ffn_beta, f)),
        w1t=C(np.asarray(ffn_w1, f).T).astype(bf),
        b1=C(np.asarray(ffn_b1, f)),
        w2t=C(np.asarray(ffn_w2, f).T).astype(bf),
        b2=C(np.asarray(ffn_b2, f)),
    )
    in_maps = [dict(base, x=C(x[b]), mask=C(mask[b])) for b in range(B)]
    return in_maps


def kernel(**inputs):
    global LAST_RESULTS
    from concourse.bass_utils import run_bass_kernel_spmd
    nc = get_nc()
    in_maps = prep_inputs(**inputs)
    res = run_bass_kernel_spmd(nc, in_maps, list(range(B)))
    LAST_RESULTS = res
    return np.stack([r["out"] for r in res.results]).astype(np.float32)


# revision 56
# speedup vs baseline: 1.0546x; 1.0546x over previous
"""nn_GBEncoderBlock on 8 TRN2 NeuronCores.

Sharding: data-parallel over batch — 1 batch element per core, SPMD, no
collectives.  Per-core layouts:
  - residual stream x: SBUF [128(l%128), 8(l//128), 512(h)] f32
  - conv/matmul operand layouts [H,L] obtained via XBAR DMA-transpose (bf16)
  - LayerNorms computed natively in [L,H] via bn_stats/bn_aggr (f32 stats)
  - depthwise conv k=7: PE matmuls with diagonal weight blocks accumulating
    the 7 taps in PSUM (LN gamma folded into the diagonals, beta+bias into
    the ACT epilogue)
  - scores computed transposed [m,l] so the key mask enters as ACT's
    per-partition bias in a fused exp(s/8 + bias); DK=64 head pairs packed
    into the PE via tile_position row strips
  - softmax rowsums via a ones-column appended to V; flash-style
    normalization after AV
All matmuls bf16 (tolerance 2e-2), psum f32, 512-wide psum tiles (1 bank).
"""

import numpy as np
import ml_dtypes

import concourse.bass as bass
from concourse import bacc
import concourse.mybir as mybir
import concourse.tile as tile

B, L, H = 8, 1024, 512
NHEAD, DK = 8, 64
KSZ, NLAYERS = 7, 4
FFN = 4 * H
EPS = 1e-6
P = 128
LT = L // P    # 8 l-tiles
HS = H // P    # 4 h-subtiles
FS = FFN // P  # 16
PAD = KSZ // 2
MASK_NEG = -30.0
LH = 512       # psum free width (one bank)
NLH = L // LH  # 2 l-halves

f32 = mybir.dt.float32
bf16 = mybir.dt.bfloat16
u8 = mybir.dt.uint8
FT = mybir.ActivationFunctionType
OP = mybir.AluOpType

# debug: "conv", "attn", "all" — where to stop emitting (sim bisection)
PHASES = "all"


def _bcast_ap(row_ap):
    """[N] DRAM AP -> [P, N] AP replicated across partitions (step-0)."""
    return bass.AP(
        tensor=row_ap.tensor,
        offset=row_ap.offset,
        ap=[[0, P]] + [list(d) for d in row_ap.ap],
    )


def _pp(vec_ap, s):
    """[s*P] DRAM AP -> [P, s] per-partition layout (h = s_idx*P + p)."""
    return vec_ap.rearrange("(s p) -> p s", p=P)


def build_nc():
    nc = bacc.Bacc()

    d = {}
    d["x_d"] = nc.dram_tensor("x", [L, H], f32, kind="ExternalInput")
    d["mask_d"] = nc.dram_tensor("mask", [L], u8, kind="ExternalInput")
    d["pos_d"] = nc.dram_tensor("pos", [L, H], f32, kind="ExternalInput")
    d["cg_d"] = nc.dram_tensor("cg", [NLAYERS, H], f32, kind="ExternalInput")
    d["cb_d"] = nc.dram_tensor("cb", [NLAYERS, H], f32, kind="ExternalInput")
    d["dww_d"] = nc.dram_tensor("dww", [NLAYERS, H, KSZ], f32, kind="ExternalInput")
    d["dwb_d"] = nc.dram_tensor("dwb", [NLAYERS, H], f32, kind="ExternalInput")
    d["pwt_d"] = nc.dram_tensor("pwt", [NLAYERS, H, H], bf16, kind="ExternalInput")
    d["pwb_d"] = nc.dram_tensor("pwb", [NLAYERS, H], f32, kind="ExternalInput")
    d["ag_d"] = nc.dram_tensor("ag", [H], f32, kind="ExternalInput")
    d["ab_d"] = nc.dram_tensor("ab", [H], f32, kind="ExternalInput")
    d["wq_d"] = nc.dram_tensor("wq", [H, H], bf16, kind="ExternalInput")
    d["wk_d"] = nc.dram_tensor("wk", [H, H], bf16, kind="ExternalInput")
    d["wv_d"] = nc.dram_tensor("wv", [H, H], bf16, kind="ExternalInput")
    d["pjt_d"] = nc.dram_tensor("pjt", [H, H], bf16, kind="ExternalInput")
    d["pjb_d"] = nc.dram_tensor("pjb", [H], f32, kind="ExternalInput")
    d["fg_d"] = nc.dram_tensor("fg", [H], f32, kind="ExternalInput")
    d["fb_d"] = nc.dram_tensor("fb", [H], f32, kind="ExternalInput")
    d["w1t_d"] = nc.dram_tensor("w1t", [H, FFN], bf16, kind="ExternalInput")
    d["b1_d"] = nc.dram_tensor("b1", [FFN], f32, kind="ExternalInput")
    d["w2t_d"] = nc.dram_tensor("w2t", [FFN, H], bf16, kind="ExternalInput")
    d["b2_d"] = nc.dram_tensor("b2", [H], f32, kind="ExternalInput")
    d["out_d"] = nc.dram_tensor("out", [L, H], f32, kind="ExternalOutput")

    with tile.TileContext(nc) as tc:
        with (
            tc.tile_pool(name="persist", bufs=1) as pp,
            tc.tile_pool(name="w8", bufs=7) as w8,
            tc.tile_pool(name="p16", bufs=2) as p16,
            tc.tile_pool(name="small", bufs=3) as sm,
            tc.tile_pool(name="psum", bufs=1, space="PSUM") as psp,
        ):
            d.update(pp=pp, w8=w8, p16=p16, sm=sm, psp=psp)
            emit(nc, d)
    nc.finalize()
    return nc


def emit(nc, env):
    pp, w8, p16, sm, psp = (
        env["pp"], env["w8"], env["p16"], env["sm"], env["psp"])
    x_d, mask_d, pos_d = env["x_d"], env["mask_d"], env["pos_d"]
    cg_d, cb_d = env["cg_d"], env["cb_d"]
    dww_d, dwb_d, pwt_d, pwb_d = env["dww_d"], env["dwb_d"], env["pwt_d"], env["pwb_d"]
    ag_d, ab_d = env["ag_d"], env["ab_d"]
    wq_d, wk_d, wv_d = env["wq_d"], env["wk_d"], env["wv_d"]
    pjt_d, pjb_d = env["pjt_d"], env["pjb_d"]
    fg_d, fb_d = env["fg_d"], env["fb_d"]
    w1t_d, b1_d, w2t_d, b2_d = env["w1t_d"], env["b1_d"], env["w2t_d"], env["b2_d"]
    out_d = env["out_d"]

    ts = bass.ts

    def psum_sc(name):
        return psp.tile([P, LH], f32, name=name, tag="sc", bufs=5)

    # ---------------- constants / persistent loads ----------------
    x_sb = pp.tile([P, LT, H], f32, name="x_sb")
    nc.sync.dma_start(out=x_sb, in_=x_d[:, :].rearrange("(lo p) h -> p lo h", p=P))

    pos_r = pos_d[:, :].rearrange("(lo p) h -> p lo h", p=P)

    mask_u = pp.tile([P, LT], u8, name="mask_u")
    nc.gpsimd.dma_start(out=mask_u, in_=mask_d[:].rearrange("(mo p) -> p mo", p=P))
    maskb = pp.tile([P, LT], f32, name="maskb")
    nc.vector.tensor_scalar_mul(maskb, mask_u, MASK_NEG)

    dww_sb = pp.tile([P, NLAYERS, HS, KSZ], f32, name="dww_sb")
    dwb_sb = pp.tile([P, NLAYERS, HS], f32, name="dwb_sb")
    pwb_sb = pp.tile([P, NLAYERS, HS], f32, name="pwb_sb")
    for i in range(NLAYERS):
        nc.sync.dma_start(
            out=dww_sb[:, i], in_=dww_d[i, :, :].rearrange("(s p) k -> p s k", p=P))
        nc.gpsimd.dma_start(out=dwb_sb[:, i], in_=_pp(dwb_d[i, :], HS))
        nc.gpsimd.dma_start(out=pwb_sb[:, i], in_=_pp(pwb_d[i, :], HS))
    pjb_sb = pp.tile([P, HS], f32, name="pjb_sb")
    nc.gpsimd.dma_start(out=pjb_sb, in_=_pp(pjb_d[:], HS))
    b1_sb = pp.tile([P, FS], f32, name="b1_sb")
    nc.gpsimd.dma_start(out=b1_sb, in_=_pp(b1_d[:], FS))
    b2_sb = pp.tile([P, HS], f32, name="b2_sb")
    nc.gpsimd.dma_start(out=b2_sb, in_=_pp(b2_d[:], HS))

    # x += pos via accumulate-DMA
    nc.gpsimd.dma_start(out=x_sb, in_=pos_r, accum_op=OP.add)

    # LN gamma/beta in per-partition [H,*] layout: conv pairs fold into the
    # depthwise diagonals; attn/ffn pairs apply on the transposed z tiles
    gbx = pp.tile([P, 2, 2, HS], f32, name="gbx")
    for j, row in enumerate((ag_d[:], ab_d[:], fg_d[:], fb_d[:])):
        nc.gpsimd.dma_start(out=gbx[:, j // 2, j % 2], in_=_pp(row, HS))

    # conv-LN gamma/beta (folded into depthwise)
    gpp_sb = pp.tile([P, NLAYERS, HS], f32, name="gpp_sb")
    bpp_sb = pp.tile([P, NLAYERS, HS], f32, name="bpp_sb")
    for i in range(NLAYERS):
        nc.gpsimd.dma_start(out=gpp_sb[:, i], in_=_pp(cg_d[i, :], HS))
        nc.gpsimd.dma_start(out=bpp_sb[:, i], in_=_pp(cb_d[i, :], HS))

    from concourse.masks import make_identity
    ident = pp.tile([P, P], bf16, name="ident")
    make_identity(nc, ident)

    def layer_norm(nm):
        """LN over H of x_sb -> z_lh [P, LT, H] bf16, gamma/beta applied
        downstream in the transposed layout (torch-style unbiased std)."""
        stats = sm.tile([P, LT, 6], f32, name="st" + nm, tag="stats")
        mv = sm.tile([P, LT, 2], f32, name="mv" + nm, tag="mv")
        for lo in range(LT):
            nc.vector.bn_stats(out=stats[:, lo], in_=x_sb[:, lo])
            nc.vector.bn_aggr(out=mv[:, lo], in_=stats[:, lo])
        std = sm.tile([P, LT], f32, name="sd" + nm, tag="std")
        nc.scalar.activation(out=std, in_=mv[:, :, 1], func=FT.Sqrt,
                             scale=float(H) / (H - 1))
        nc.vector.tensor_scalar_add(std, std, EPS)
        rstd = sm.tile([P, LT], f32, name="rs" + nm, tag="rstd")
        nc.vector.reciprocal(out=rstd, in_=std)
        bm = sm.tile([P, LT], f32, name="bm" + nm, tag="bm")
        nc.vector.tensor_mul(bm, mv[:, :, 0], rstd)
        nc.vector.tensor_scalar_mul(bm, bm, -1.0)
        z_lh = w8.tile([P, LT, H], bf16, name="zlh" + nm, tag="w8")
        for lo in range(LT):
            nc.scalar.activation(
                out=z_lh[:, lo], in_=x_sb[:, lo], func=FT.Identity,
                scale=rstd[:, lo:lo + 1], bias=bm[:, lo:lo + 1])
        return z_lh

    def transpose_lh_to_int(z_lh, nm, gb_j):
        """[P,LT,H] -> two half XBAR transposes -> [P(h%128), 4, HS, P(l%128)]
        per l-half, then gamma/beta (per-partition here) per h-subtile.
        Per-half tiles let QKV/W1 start after half the LN applies."""
        LTH = LT // NLH
        zts = []
        for lh in range(NLH):
            zt = sm.tile([P, LTH, HS, P], bf16, name=f"zint{nm}{lh}",
                         tag="acch", bufs=4)
            nc.sync.dma_start_transpose(
                zt, z_lh[:, lh * LTH:(lh + 1) * LTH, :])
            for ks in range(HS):
                nc.vector.tensor_scalar(
                    out=zt[:, :, ks, :], in0=zt[:, :, ks, :],
                    scalar1=gbx[:, gb_j, 0, ks:ks + 1],
                    scalar2=gbx[:, gb_j, 1, ks:ks + 1],
                    op0=OP.mult, op1=OP.add)
            zts.append(zt)
        return zts

    # ---------------- conv layers ----------------
    for i in range(NLAYERS):
        # depthwise as PE diag-matmuls; LN gamma folds into the diagonals,
        # (LN beta + dw bias) folds into the psum epilogue bias:
        #   dw(g*z+b) = sum_k w_k*g (.) shift_k(z) + (dwb + b*sum_k w_k)
        wg = sm.tile([P, HS, KSZ], f32, name=f"wg{i}", tag="wg", bufs=2)
        nc.vector.tensor_mul(
            wg, dww_sb[:, i],
            gpp_sb[:, i].unsqueeze(-1).to_broadcast([P, HS, KSZ]))
        dmat = w8.tile([P, HS, KSZ, P], bf16, name=f"dmat{i}", tag="w8")
        nc.vector.tensor_mul(
            dmat,
            ident.unsqueeze(1).unsqueeze(1).to_broadcast([P, HS, KSZ, P]),
            wg.unsqueeze(-1).to_broadcast([P, HS, KSZ, P]))
        swk = sm.tile([P, HS], f32, name=f"swk{i}", tag="swk", bufs=2)
        nc.vector.tensor_reduce(out=swk, in_=dww_sb[:, i],
                                axis=mybir.AxisListType.X, op=OP.add)
        bprime = sm.tile([P, HS], f32, name=f"bp{i}", tag="bprime", bufs=2)
        nc.vector.tensor_mul(bprime, swk, bpp_sb[:, i])
        nc.vector.tensor_add(bprime, bprime, dwb_sb[:, i])

        # LN stats
        stats = sm.tile([P, LT, 6], f32, name=f"stc{i}", tag="stats")
        mv = sm.tile([P, LT, 2], f32, name=f"mvc{i}", tag="mv")
        for lo in range(LT):
            nc.vector.bn_stats(out=stats[:, lo], in_=x_sb[:, lo])
            nc.vector.bn_aggr(out=mv[:, lo], in_=stats[:, lo])
        std = sm.tile([P, LT], f32, name=f"sdc{i}", tag="std")
        nc.scalar.activation(out=std, in_=mv[:, :, 1], func=FT.Sqrt,
                             scale=float(H) / (H - 1))
        nc.vector.tensor_scalar_add(std, std, EPS)
        rstd = sm.tile([P, LT], f32, name=f"rsc{i}", tag="rstd")
        nc.vector.reciprocal(out=rstd, in_=std)
        bm = sm.tile([P, LT], f32, name=f"bmc{i}", tag="bm")
        nc.vector.tensor_mul(bm, mv[:, :, 0], rstd)
        nc.vector.tensor_scalar_mul(bm, bm, -1.0)

        pwt_sb = w8.tile([P, HS, H], bf16, name=f"pwt{i}", tag="w8")
        nc.sync.dma_start(
            out=pwt_sb, in_=pwt_d[i, :, :].rearrange("(s p) o -> p s o", p=P))

        z_lh = w8.tile([P, LT, H], bf16, name=f"zlhc{i}", tag="w8")
        # per-l-half padded [H,L] tiles so dw(lh) starts as soon as its half
        # (plus the 3-col halo) is transposed
        zh = [w8.tile([P, HS, LH + 2 * PAD], bf16, name=f"zhl{i}_{j}",
                      tag="w8p", bufs=4) for j in range(NLH)]
        nc.vector.memset(zh[0][:, :, 0:PAD], 0.0)
        nc.vector.memset(zh[1][:, :, LH + PAD:LH + 2 * PAD], 0.0)
        ach = [sm.tile([P, HS, LH], bf16, name=f"acc{i}_{j}", tag="acch",
                       bufs=4) for j in range(NLH)]
        hh = [sm.tile([P, HS, LH], bf16, name=f"hhl{i}_{j}", tag="acch",
                      bufs=4) for j in range(NLH)]

        def conv_apply_transpose(lo):
            nc.scalar.activation(
                out=z_lh[:, lo], in_=x_sb[:, lo], func=FT.Identity,
                scale=rstd[:, lo:lo + 1], bias=bm[:, lo:lo + 1])
            ztmp = sm.tile([P, HS, P], bf16, name=f"ztmp{i}_{lo}", tag="ztmp",
                           bufs=4)
            nc.sync.dma_start_transpose(ztmp, z_lh[:, lo, :])
            j, c = lo // 4, (lo % 4) * P
            nc.vector.tensor_copy(out=zh[j][:, :, PAD + c:PAD + c + P],
                                  in_=ztmp)
            if lo == 4:  # halo: first PAD cols of lh1 -> right edge of lh0
                nc.vector.tensor_copy(out=zh[0][:, :, PAD + LH:],
                                      in_=ztmp[:, :, 0:PAD])
            if lo == 3:  # halo: last PAD cols of lh0 -> left edge of lh1
                nc.vector.tensor_copy(out=zh[1][:, :, 0:PAD],
                                      in_=ztmp[:, :, P - PAD:P])

        def conv_dw(lh):
            # depthwise on PE: psum[c',l] += D_k[c,c']z[c,l+k], D_k=diag(w_k*g)
            # split into two 64x64 diagonal quadrants packed at (0,0)/(64,64)
            for s in range(HS):
                ps = psum_sc(f"dwps{i}_{s}_{lh}")
                for k in range(KSZ):
                    nc.tensor.matmul(
                        ps, dmat[:, s, k, :], zh[lh][:, s, k:k + LH],
                        start=(k == 0), stop=(k == KSZ - 1))
                nc.scalar.activation(
                    out=ach[lh][:, s], in_=ps,
                    func=FT.Identity, bias=bprime[:, s:s + 1], scale=1.0)

        def conv_pw(lh):
            for ot in range(HS):
                ps = psum_sc(f"pwps{i}_{ot}_{lh}")
                for ks in range(HS):
                    nc.tensor.matmul(
                        ps, pwt_sb[:, ks, ts(ot, P)], ach[lh][:, ks],
                        start=(ks == 0), stop=(ks == HS - 1))
                nc.scalar.activation(
                    out=hh[lh][:, ot], in_=ps,
                    func=FT.Relu, bias=pwb_sb[:, i, ot:ot + 1], scale=1.0)

        def conv_tail(lh):
            ht = sm.tile([P, HS, 4, P], bf16, name=f"ht{i}_{lh}", tag="acch",
                         bufs=4)
            nc.sync.dma_start_transpose(ht, hh[lh][:, :, :])
            for lo in range(4 * lh, 4 * lh + 4):
                xv = x_sb[:, lo].rearrange("p (a b) -> p a b", b=P)
                eng = nc.gpsimd if lo % 3 == 2 else nc.vector
                eng.tensor_add(xv, xv, ht[:, :, lo % 4, :])

        # pipeline the two l-halves
        for lo in range(5):
            conv_apply_transpose(lo)
        conv_dw(0)
        for lo in range(5, LT):
            conv_apply_transpose(lo)
        conv_pw(0)
        conv_dw(1)
        conv_tail(0)
        conv_pw(1)
        conv_tail(1)

    if PHASES == "conv":
        nc.sync.dma_start(
            out=out_d[:, :].rearrange("(lo p) h -> p lo h", p=P), in_=x_sb)
        return

    # ---------------- attention ----------------
    z_lh = layer_norm("a")
    zq_t = transpose_lh_to_int(z_lh, "a", 0)  # [P(h), LT, HS, P(l)]

    # per-headpair q/k/v tiles, emitted PAIR-major so scores(hp0) starts
    # after 3 projection groups instead of 9; vt2 per-pair so AV(hp) only
    # waits its own pair's V rearrangement
    q_t, k_t, v_t = ({}, {}, {})
    vt_r, vt2 = ({}, {})
    w_sbs = {}
    for wname, w_d in (("q", wq_d), ("k", wk_d), ("v", wv_d)):
        w_sb = w8.tile([P, HS, H], bf16, name=f"w{wname}sb", tag="w8")
        nc.sync.dma_start(out=w_sb, in_=w_d[:, :].rearrange("(s p) o -> p s o", p=P))
        w_sbs[wname] = w_sb
    for ot in range(HS):
        for wname, store in (("q", q_t), ("k", k_t), ("v", v_t)):
            prj = sm.tile([P, L], bf16, name=f"{wname}sb{ot}", tag="qkv",
                          bufs=18)
            store[ot] = prj
            for lh in range(NLH):
                ps = psum_sc(f"qkvps{wname}{ot}_{lh}")
                for ks in range(HS):
                    nc.tensor.matmul(
                        ps, w_sbs[wname][:, ks, ts(ot, P)],
                        zq_t[lh][:, :, ks, :],
                        start=(ks == 0), stop=(ks == HS - 1))
                dst = prj[:, lh * LH:(lh + 1) * LH]
                if wname == "v":
                    nc.scalar.copy(out=dst, in_=ps)
                else:
                    nc.vector.tensor_copy(out=dst, in_=ps)
        # V^T per pair: [dv(2 heads), m] -> [m, dv], plus ones column
        vr = sm.tile([P, LT, P], bf16, name=f"vtr{ot}", tag="qkv", bufs=18)
        nc.sync.dma_start_transpose(vr, v_t[ot][:, :])
        vt_r[ot] = vr
        v2 = sm.tile([P, LT, 2, DK + 1], bf16, name=f"vt2_{ot}", tag="qkv",
                     bufs=18)
        for mo in range(LT):
            nc.vector.tensor_copy(out=v2[:, mo, 0, 0:DK],
                                  in_=vr[:, mo, 0:DK])
            nc.vector.tensor_copy(out=v2[:, mo, 1, 0:DK],
                                  in_=vr[:, mo, DK:2 * DK])
        nc.vector.memset(v2[:, :, :, DK:DK + 1], 1.0)
        vt2[ot] = v2

    oT_sb = w8.tile([P, HS, L], bf16, name="oT_sb", tag="w8")
    ones1 = pp.tile([P, DK], bf16, name="ones1")
    nc.vector.memset(ones1, 1.0)
    sc_scale = 1.0 / float(np.sqrt(DK))
    for hp in range(HS):
        hA, hB = 2 * hp, 2 * hp + 1
        pA = [p16.tile([P, LT, LH], bf16, name=f"pA{hp}_{j}", tag="p16",
                       bufs=4) for j in range(NLH)]
        pB = [p16.tile([P, LT, LH], bf16, name=f"pB{hp}_{j}", tag="p16",
                       bufs=4) for j in range(NLH)]
        for lh in range(NLH):
            for mo in range(LT):
                psA = psum_sc(f"sA{hp}_{mo}_{lh}")
                psB = psum_sc(f"sB{hp}_{mo}_{lh}")
                nc.tensor.matmul(
                    psA, k_t[hp][0:DK, ts(mo, P)],
                    q_t[hp][0:DK, lh * LH:(lh + 1) * LH],
                    start=True, stop=True, tile_position=(0, 0))
                nc.tensor.matmul(
                    psB, k_t[hp][DK:P, ts(mo, P)],
                    q_t[hp][DK:P, lh * LH:(lh + 1) * LH],
                    start=True, stop=True, tile_position=(DK, 0))
                nc.scalar.activation(
                    out=pA[lh][:, mo, :], in_=psA, func=FT.Exp,
                    bias=maskb[:, mo:mo + 1], scale=sc_scale)
                nc.scalar.activation(
                    out=pB[lh][:, mo, :], in_=psB, func=FT.Exp,
                    bias=maskb[:, mo:mo + 1], scale=sc_scale)

        for (hh, ph, part0) in ((hA, pA, True), (hB, pB, False)):
            rb = sm.tile([P, L], bf16, name=f"rb{hh}", tag="rbc", bufs=2)
            if not part0:
                otmp = sm.tile([DK, L], bf16, name=f"ot{hh}", tag="otmp", bufs=2)
            for lhx in range(NLH):
                lsl = slice(lhx * LH, (lhx + 1) * LH)
                rtmp = sm.tile([P, LH], bf16, name=f"rt{hh}_{lhx}", tag="rtmp",
                               bufs=2)
                r0 = sm.tile([P, LH], bf16, name=f"r0{hh}_{lhx}", tag="r0",
                             bufs=2)
                pso = psp.tile([DK + 1, LH], f32, name=f"av{hh}_{lhx}",
                               tag="av", bufs=3)
                for mo in range(LT):
                    nc.tensor.matmul(pso, vt2[hp][:, mo, hh % 2, 0:DK + 1],
                                     ph[lhx][:, mo, :],
                                     start=(mo == 0), stop=(mo == LT - 1))
                # reciprocal of rowsum (lives at partition DK=64)
                with nc.allow_low_precision(reason="softmax denom in bf16"):
                    nc.vector.reciprocal(out=rtmp[DK:DK + 1, :],
                                         in_=pso[DK:DK + 1, :])
                nc.sync.dma_start(out=r0[0:1, :], in_=rtmp[DK:DK + 1, :])
                # broadcast across DK partitions: ones [1,DK] outer r0 [1,LH]
                psR = psum_sc(f"psR{hh}_{lhx}")
                nc.tensor.matmul(psR[0:DK, :], ones1[0:1, :], r0[0:1, :],
                                 start=True, stop=True)
                nc.vector.tensor_copy(out=rb[0:DK, lsl], in_=psR[0:DK, :])
                if part0:
                    nc.vector.scalar_tensor_tensor(
                        out=oT_sb[0:DK, hp, lsl], in0=pso[0:DK, :], scalar=0.0,
                        in1=rb[0:DK, lsl], op0=OP.bypass, op1=OP.mult)
                else:
                    nc.vector.scalar_tensor_tensor(
                        out=otmp[:, lsl], in0=pso[0:DK, :], scalar=0.0,
                        in1=rb[0:DK, lsl], op0=OP.bypass, op1=OP.mult)
            if not part0:
                nc.sync.dma_start(out=oT_sb[DK:P, hp, :], in_=otmp)

    # output projection + residual (per l-half tails)
    pjt_sb = w8.tile([P, HS, H], bf16, name="pjt_sb", tag="w8")
    nc.sync.dma_start(out=pjt_sb, in_=pjt_d[:, :].rearrange("(s p) o -> p s o", p=P))
    for lh in range(NLH):
        pr_h = sm.tile([P, HS, LH], bf16, name=f"prh{lh}", tag="acch", bufs=4)
        for ot in range(HS):
            ps = psum_sc(f"prps{ot}_{lh}")
            for ds in range(HS):
                nc.tensor.matmul(ps, pjt_sb[:, ds, ts(ot, P)],
                                 oT_sb[:, ds, lh * LH:(lh + 1) * LH],
                                 start=(ds == 0), stop=(ds == HS - 1))
            nc.scalar.activation(
                out=pr_h[:, ot], in_=ps,
                func=FT.Identity, bias=pjb_sb[:, ot:ot + 1], scale=1.0)
        prt = sm.tile([P, HS, 4, P], bf16, name=f"prt{lh}", tag="acch", bufs=4)
        nc.sync.dma_start_transpose(prt, pr_h[:, :, :])
        for lo in range(4 * lh, 4 * lh + 4):
            xv = x_sb[:, lo].rearrange("p (a b) -> p a b", b=P)
            eng = nc.gpsimd if lo % 3 == 2 else nc.vector
            eng.tensor_add(xv, xv, prt[:, :, lo % 4, :])

    if PHASES == "attn":
        nc.sync.dma_start(
            out=out_d[:, :].rearrange("(lo p) h -> p lo h", p=P), in_=x_sb)
        return

    # ---------------- FFN ----------------
    z_lh = layer_norm("f")
    zf_t = transpose_lh_to_int(z_lh, "f", 1)

    # h1 in 4 quarter tiles so W2's accumulation starts after each quarter
    h1q = [w8.tile([P, FS // 4, L], bf16, name=f"h1q{j}", tag="w8")
           for j in range(4)]
    w2_sb = w8.tile([P, FS, H], bf16, name="w2_sb", tag="w2", bufs=1)
    nc.sync.dma_start(out=w2_sb, in_=w2t_d[:, :].rearrange("(s p) o -> p s o", p=P))
    w1h = [w8.tile([P, 2, FFN], bf16, name=f"w1h{j}", tag="w8")
           for j in range(2)]
    for j in range(2):
        nc.sync.dma_start(
            out=w1h[j],
            in_=w1t_d[:, :].rearrange("(s p) o -> p s o", p=P)[:, 2 * j:2 * j + 2])
    for ot in range(FS):
        htile = h1q[ot // (FS // 4)]
        for lh in range(NLH):
            ps = psum_sc(f"f1ps{ot}_{lh}")
            for ks in range(HS):
                nc.tensor.matmul(
                    ps, w1h[ks // 2][:, ks % 2, ts(ot, P)],
                    zf_t[lh][:, :, ks, :],
                    start=(ks == 0), stop=(ks == HS - 1))
            nc.scalar.activation(
                out=htile[:, ot % (FS // 4), lh * LH:(lh + 1) * LH], in_=ps,
                func=FT.Relu, bias=b1_sb[:, ot:ot + 1], scale=1.0)

    for lh in range(NLH):
        o2_h = sm.tile([P, HS, LH], bf16, name=f"o2h{lh}", tag="acch", bufs=4)
        for ot in range(HS):
            ps = psum_sc(f"f2ps{ot}_{lh}")
            for ks in range(FS):
                htile = h1q[ks // (FS // 4)]
                nc.tensor.matmul(ps, w2_sb[:, ks, ts(ot, P)],
                                 htile[:, ks % (FS // 4), lh * LH:(lh + 1) * LH],
                                 start=(ks == 0), stop=(ks == FS - 1))
            nc.scalar.activation(
                out=o2_h[:, ot], in_=ps,
                func=FT.Identity, bias=b2_sb[:, ot:ot + 1], scale=1.0)
        o2t = sm.tile([P, HS, 4, P], bf16, name=f"o2t{lh}", tag="acch", bufs=4)
        nc.sync.dma_start_transpose(o2t, o2_h[:, :, :])
        for lo in range(4 * lh, 4 * lh + 4):
            xv = x_sb[:, lo].rearrange("p (a b) -> p a b", b=P)
            eng = nc.gpsimd if lo % 3 == 2 else nc.vector
            eng.tensor_add(xv, xv, o2t[:, :, lo % 4, :])

    nc.sync.dma_start(
        out=out_d[:, :].rearrange("(lo p) h -> p lo h", p=P), in_=x_sb)


_NC_CACHE = {}
LAST_RESULTS = None


def get_nc():
    if "nc" not in _NC_CACHE:
        _NC_CACHE["nc"] = build_nc()
    return _NC_CACHE["nc"]


def prep_inputs(x, x_mask, pos_emb, cnn_gamma, cnn_beta, cnn_dw_w, cnn_dw_b,
                cnn_pw_w, cnn_pw_b, attn_gamma, attn_beta, w_qs, w_ks, w_vs,
                proj_w, proj_b, ffn_gamma, ffn_beta, ffn_w1, ffn_b1,
                ffn_w2, ffn_b2):
    bf = ml_dtypes.bfloat16
    f = np.float32
    C = np.ascontiguousarray

    x = np.asarray(x, f)
    mask = np.asarray(x_mask).astype(np.uint8)
    base = dict(
        pos=C(np.asarray(pos_emb, f)[0, :L, :]),
        cg=C(np.asarray(cnn_gamma, f)), cb=C(np.asarray(cnn_beta, f)),
        dww=C(np.asarray(cnn_dw_w, f)), dwb=C(np.asarray(cnn_dw_b, f)),
        pwt=C(np.transpose(np.asarray(cnn_pw_w, f), (0, 2, 1))).astype(bf),
        pwb=C(np.asarray(cnn_pw_b, f)),
        ag=C(np.asarray(attn_gamma, f)), ab=C(np.asarray(attn_beta, f)),
        wq=C(np.transpose(np.asarray(w_qs, f), (1, 0, 2)).reshape(H, H)).astype(bf),
        wk=C(np.transpose(np.asarray(w_ks, f), (1, 0, 2)).reshape(H, H)).astype(bf),
        wv=C(np.transpose(np.asarray(w_vs, f), (1, 0, 2)).reshape(H, H)).astype(bf),
        pjt=C(np.asarray(proj_w, f).T).astype(bf),
        pjb=C(np.asarray(proj_b, f)),
        fg=C(np.asarray(ffn_gamma, f)), fb=C(np.asarray(ffn_beta, f)),
        w1t=C(np.asarray(ffn_w1, f).T).astype(bf),
        b1=C(np.asarray(ffn_b1, f)),
        w2t=C(np.asarray(ffn_w2, f).T).astype(bf),
        b2=C(np.asarray(ffn_b2, f)),
    )
    in_maps = [dict(base, x=C(x[b]), mask=C(mask[b])) for b in range(B)]
    return in_maps


def kernel(**inputs):
    global LAST_RESULTS
    from concourse.bass_utils import run_bass_kernel_spmd
    nc = get_nc()
    in_maps = prep_inputs(**inputs)
    res = run_bass_kernel_spmd(nc, in_maps, list(range(B)))
    LAST_RESULTS = res
    return np.stack([r["out"] for r in res.results]).astype(np.float32)


# revision 58
# speedup vs baseline: 1.0691x; 1.0138x over previous
"""nn_GBEncoderBlock on 8 TRN2 NeuronCores.

Sharding: data-parallel over batch — 1 batch element per core, SPMD, no
collectives.  Per-core layouts:
  - residual stream x: SBUF [128(l%128), 8(l//128), 512(h)] f32
  - conv/matmul operand layouts [H,L] obtained via XBAR DMA-transpose (bf16)
  - LayerNorms computed natively in [L,H] via bn_stats/bn_aggr (f32 stats)
  - depthwise conv k=7: PE matmuls with diagonal weight blocks accumulating
    the 7 taps in PSUM (LN gamma folded into the diagonals, beta+bias into
    the ACT epilogue)
  - scores computed transposed [m,l] so the key mask enters as ACT's
    per-partition bias in a fused exp(s/8 + bias); DK=64 head pairs packed
    into the PE via tile_position row strips
  - softmax rowsums via a ones-column appended to V; flash-style
    normalization after AV
All matmuls bf16 (tolerance 2e-2), psum f32, 512-wide psum tiles (1 bank).
"""

import numpy as np
import ml_dtypes

import concourse.bass as bass
from concourse import bacc
import concourse.mybir as mybir
import concourse.tile as tile

B, L, H = 8, 1024, 512
NHEAD, DK = 8, 64
KSZ, NLAYERS = 7, 4
FFN = 4 * H
EPS = 1e-6
P = 128
LT = L // P    # 8 l-tiles
HS = H // P    # 4 h-subtiles
FS = FFN // P  # 16
PAD = KSZ // 2
MASK_NEG = -30.0
LH = 512       # psum free width (one bank)
NLH = L // LH  # 2 l-halves

f32 = mybir.dt.float32
bf16 = mybir.dt.bfloat16
u8 = mybir.dt.uint8
FT = mybir.ActivationFunctionType
OP = mybir.AluOpType

# debug: "conv", "attn", "all" — where to stop emitting (sim bisection)
PHASES = "all"


def _bcast_ap(row_ap):
    """[N] DRAM AP -> [P, N] AP replicated across partitions (step-0)."""
    return bass.AP(
        tensor=row_ap.tensor,
        offset=row_ap.offset,
        ap=[[0, P]] + [list(d) for d in row_ap.ap],
    )


def _pp(vec_ap, s):
    """[s*P] DRAM AP -> [P, s] per-partition layout (h = s_idx*P + p)."""
    return vec_ap.rearrange("(s p) -> p s", p=P)


def build_nc():
    nc = bacc.Bacc()

    d = {}
    d["x_d"] = nc.dram_tensor("x", [L, H], f32, kind="ExternalInput")
    d["mask_d"] = nc.dram_tensor("mask", [L], u8, kind="ExternalInput")
    d["pos_d"] = nc.dram_tensor("pos", [L, H], f32, kind="ExternalInput")
    d["cg_d"] = nc.dram_tensor("cg", [NLAYERS, H], f32, kind="ExternalInput")
    d["cb_d"] = nc.dram_tensor("cb", [NLAYERS, H], f32, kind="ExternalInput")
    d["dww_d"] = nc.dram_tensor("dww", [NLAYERS, H, KSZ], f32, kind="ExternalInput")
    d["dwb_d"] = nc.dram_tensor("dwb", [NLAYERS, H], f32, kind="ExternalInput")
    d["pwt_d"] = nc.dram_tensor("pwt", [NLAYERS, H, H], bf16, kind="ExternalInput")
    d["pwb_d"] = nc.dram_tensor("pwb", [NLAYERS, H], f32, kind="ExternalInput")
    d["ag_d"] = nc.dram_tensor("ag", [H], f32, kind="ExternalInput")
    d["ab_d"] = nc.dram_tensor("ab", [H], f32, kind="ExternalInput")
    d["wq_d"] = nc.dram_tensor("wq", [H, H], bf16, kind="ExternalInput")
    d["wk_d"] = nc.dram_tensor("wk", [H, H], bf16, kind="ExternalInput")
    d["wv_d"] = nc.dram_tensor("wv", [H, H], bf16, kind="ExternalInput")
    d["pjt_d"] = nc.dram_tensor("pjt", [H, H], bf16, kind="ExternalInput")
    d["pjb_d"] = nc.dram_tensor("pjb", [H], f32, kind="ExternalInput")
    d["fg_d"] = nc.dram_tensor("fg", [H], f32, kind="ExternalInput")
    d["fb_d"] = nc.dram_tensor("fb", [H], f32, kind="ExternalInput")
    d["w1t_d"] = nc.dram_tensor("w1t", [H, FFN], bf16, kind="ExternalInput")
    d["b1_d"] = nc.dram_tensor("b1", [FFN], f32, kind="ExternalInput")
    d["w2t_d"] = nc.dram_tensor("w2t", [FFN, H], bf16, kind="ExternalInput")
    d["b2_d"] = nc.dram_tensor("b2", [H], f32, kind="ExternalInput")
    d["out_d"] = nc.dram_tensor("out", [L, H], f32, kind="ExternalOutput")

    with tile.TileContext(nc) as tc:
        with (
            tc.tile_pool(name="persist", bufs=1) as pp,
            tc.tile_pool(name="w8", bufs=7) as w8,
            tc.tile_pool(name="p16", bufs=2) as p16,
            tc.tile_pool(name="small", bufs=3) as sm,
            tc.tile_pool(name="psum", bufs=1, space="PSUM") as psp,
        ):
            d.update(pp=pp, w8=w8, p16=p16, sm=sm, psp=psp)
            emit(nc, d)
    nc.finalize()
    return nc


def emit(nc, env):
    pp, w8, p16, sm, psp = (
        env["pp"], env["w8"], env["p16"], env["sm"], env["psp"])
    x_d, mask_d, pos_d = env["x_d"], env["mask_d"], env["pos_d"]
    cg_d, cb_d = env["cg_d"], env["cb_d"]
    dww_d, dwb_d, pwt_d, pwb_d = env["dww_d"], env["dwb_d"], env["pwt_d"], env["pwb_d"]
    ag_d, ab_d = env["ag_d"], env["ab_d"]
    wq_d, wk_d, wv_d = env["wq_d"], env["wk_d"], env["wv_d"]
    pjt_d, pjb_d = env["pjt_d"], env["pjb_d"]
    fg_d, fb_d = env["fg_d"], env["fb_d"]
    w1t_d, b1_d, w2t_d, b2_d = env["w1t_d"], env["b1_d"], env["w2t_d"], env["b2_d"]
    out_d = env["out_d"]

    ts = bass.ts

    def psum_sc(name):
        return psp.tile([P, LH], f32, name=name, tag="sc", bufs=5)

    # ---------------- constants / persistent loads ----------------
    x_sb = pp.tile([P, LT, H], f32, name="x_sb")
    nc.sync.dma_start(out=x_sb, in_=x_d[:, :].rearrange("(lo p) h -> p lo h", p=P))

    pos_r = pos_d[:, :].rearrange("(lo p) h -> p lo h", p=P)

    mask_u = pp.tile([P, LT], u8, name="mask_u")
    nc.gpsimd.dma_start(out=mask_u, in_=mask_d[:].rearrange("(mo p) -> p mo", p=P))
    maskb = pp.tile([P, LT], f32, name="maskb")
    nc.vector.tensor_scalar_mul(maskb, mask_u, MASK_NEG)

    dww_sb = pp.tile([P, NLAYERS, HS, KSZ], f32, name="dww_sb")
    dwb_sb = pp.tile([P, NLAYERS, HS], f32, name="dwb_sb")
    pwb_sb = pp.tile([P, NLAYERS, HS], f32, name="pwb_sb")
    for i in range(NLAYERS):
        nc.sync.dma_start(
            out=dww_sb[:, i], in_=dww_d[i, :, :].rearrange("(s p) k -> p s k", p=P))
        nc.gpsimd.dma_start(out=dwb_sb[:, i], in_=_pp(dwb_d[i, :], HS))
        nc.gpsimd.dma_start(out=pwb_sb[:, i], in_=_pp(pwb_d[i, :], HS))
    pjb_sb = pp.tile([P, HS], f32, name="pjb_sb")
    nc.gpsimd.dma_start(out=pjb_sb, in_=_pp(pjb_d[:], HS))
    b1_sb = pp.tile([P, FS], f32, name="b1_sb")
    nc.gpsimd.dma_start(out=b1_sb, in_=_pp(b1_d[:], FS))
    b2_sb = pp.tile([P, HS], f32, name="b2_sb")
    nc.gpsimd.dma_start(out=b2_sb, in_=_pp(b2_d[:], HS))

    # x += pos via accumulate-DMA
    nc.gpsimd.dma_start(out=x_sb, in_=pos_r, accum_op=OP.add)

    # LN gamma/beta in per-partition [H,*] layout: conv pairs fold into the
    # depthwise diagonals; attn/ffn pairs apply on the transposed z tiles
    gbx = pp.tile([P, 2, 2, HS], f32, name="gbx")
    for j, row in enumerate((ag_d[:], ab_d[:], fg_d[:], fb_d[:])):
        nc.gpsimd.dma_start(out=gbx[:, j // 2, j % 2], in_=_pp(row, HS))

    # conv-LN gamma/beta (folded into depthwise)
    gpp_sb = pp.tile([P, NLAYERS, HS], f32, name="gpp_sb")
    bpp_sb = pp.tile([P, NLAYERS, HS], f32, name="bpp_sb")
    for i in range(NLAYERS):
        nc.gpsimd.dma_start(out=gpp_sb[:, i], in_=_pp(cg_d[i, :], HS))
        nc.gpsimd.dma_start(out=bpp_sb[:, i], in_=_pp(cb_d[i, :], HS))

    from concourse.masks import make_identity
    ident = pp.tile([P, P], bf16, name="ident")
    make_identity(nc, ident)

    def layer_norm(nm):
        """LN over H of x_sb -> z_lh [P, LT, H] bf16, gamma/beta applied
        downstream in the transposed layout (torch-style unbiased std)."""
        stats = sm.tile([P, LT, 6], f32, name="st" + nm, tag="stats")
        mv = sm.tile([P, LT, 2], f32, name="mv" + nm, tag="mv")
        for lo in range(LT):
            nc.vector.bn_stats(out=stats[:, lo], in_=x_sb[:, lo])
            nc.vector.bn_aggr(out=mv[:, lo], in_=stats[:, lo])
        std = sm.tile([P, LT], f32, name="sd" + nm, tag="std")
        nc.scalar.activation(out=std, in_=mv[:, :, 1], func=FT.Sqrt,
                             scale=float(H) / (H - 1))
        nc.vector.tensor_scalar_add(std, std, EPS)
        rstd = sm.tile([P, LT], f32, name="rs" + nm, tag="rstd")
        nc.vector.reciprocal(out=rstd, in_=std)
        bm = sm.tile([P, LT], f32, name="bm" + nm, tag="bm")
        nc.vector.tensor_mul(bm, mv[:, :, 0], rstd)
        nc.vector.tensor_scalar_mul(bm, bm, -1.0)
        z_lh = w8.tile([P, LT, H], bf16, name="zlh" + nm, tag="w8")
        for lo in range(LT):
            nc.scalar.activation(
                out=z_lh[:, lo], in_=x_sb[:, lo], func=FT.Identity,
                scale=rstd[:, lo:lo + 1], bias=bm[:, lo:lo + 1])
        return z_lh

    def transpose_lh_to_int(z_lh, nm, gb_j):
        """[P,LT,H] -> two half XBAR transposes -> [P(h%128), 4, HS, P(l%128)]
        per l-half, then gamma/beta (per-partition here) per h-subtile.
        Per-half tiles let QKV/W1 start after half the LN applies."""
        LTH = LT // NLH
        zts = []
        for lh in range(NLH):
            zt = sm.tile([P, LTH, HS, P], bf16, name=f"zint{nm}{lh}",
                         tag="acch", bufs=4)
            nc.sync.dma_start_transpose(
                zt, z_lh[:, lh * LTH:(lh + 1) * LTH, :])
            for ks in range(HS):
                nc.vector.tensor_scalar(
                    out=zt[:, :, ks, :], in0=zt[:, :, ks, :],
                    scalar1=gbx[:, gb_j, 0, ks:ks + 1],
                    scalar2=gbx[:, gb_j, 1, ks:ks + 1],
                    op0=OP.mult, op1=OP.add)
            zts.append(zt)
        return zts

    # ---------------- conv layers ----------------
    for i in range(NLAYERS):
        # depthwise as PE diag-matmuls; LN gamma folds into the diagonals,
        # (LN beta + dw bias) folds into the psum epilogue bias:
        #   dw(g*z+b) = sum_k w_k*g (.) shift_k(z) + (dwb + b*sum_k w_k)
        wg = sm.tile([P, HS, KSZ], f32, name=f"wg{i}", tag="wg", bufs=2)
        nc.vector.tensor_mul(
            wg, dww_sb[:, i],
            gpp_sb[:, i].unsqueeze(-1).to_broadcast([P, HS, KSZ]))
        dmat = w8.tile([P, HS, KSZ, P], bf16, name=f"dmat{i}", tag="w8")
        nc.vector.tensor_mul(
            dmat,
            ident.unsqueeze(1).unsqueeze(1).to_broadcast([P, HS, KSZ, P]),
            wg.unsqueeze(-1).to_broadcast([P, HS, KSZ, P]))
        swk = sm.tile([P, HS], f32, name=f"swk{i}", tag="swk", bufs=2)
        nc.vector.tensor_reduce(out=swk, in_=dww_sb[:, i],
                                axis=mybir.AxisListType.X, op=OP.add)
        bprime = sm.tile([P, HS], f32, name=f"bp{i}", tag="bprime", bufs=2)
        nc.vector.tensor_mul(bprime, swk, bpp_sb[:, i])
        nc.vector.tensor_add(bprime, bprime, dwb_sb[:, i])

        # LN stats
        stats = sm.tile([P, LT, 6], f32, name=f"stc{i}", tag="stats")
        mv = sm.tile([P, LT, 2], f32, name=f"mvc{i}", tag="mv")
        for lo in range(LT):
            nc.vector.bn_stats(out=stats[:, lo], in_=x_sb[:, lo])
            nc.vector.bn_aggr(out=mv[:, lo], in_=stats[:, lo])
        std = sm.tile([P, LT], f32, name=f"sdc{i}", tag="std")
        nc.scalar.activation(out=std, in_=mv[:, :, 1], func=FT.Sqrt,
                             scale=float(H) / (H - 1))
        nc.vector.tensor_scalar_add(std, std, EPS)
        rstd = sm.tile([P, LT], f32, name=f"rsc{i}", tag="rstd")
        nc.vector.reciprocal(out=rstd, in_=std)
        bm = sm.tile([P, LT], f32, name=f"bmc{i}", tag="bm")
        nc.vector.tensor_mul(bm, mv[:, :, 0], rstd)
        nc.vector.tensor_scalar_mul(bm, bm, -1.0)

        pwt_sb = w8.tile([P, HS, H], bf16, name=f"pwt{i}", tag="w8")
        nc.sync.dma_start(
            out=pwt_sb, in_=pwt_d[i, :, :].rearrange("(s p) o -> p s o", p=P))

        z_lh = w8.tile([P, LT, H], bf16, name=f"zlhc{i}", tag="w8")
        # per-l-half padded [H,L] tiles so dw(lh) starts as soon as its half
        # (plus the 3-col halo) is transposed
        zh = [w8.tile([P, HS, LH + 2 * PAD], bf16, name=f"zhl{i}_{j}",
                      tag="w8p", bufs=4) for j in range(NLH)]
        nc.vector.memset(zh[0][:, :, 0:PAD], 0.0)
        nc.vector.memset(zh[1][:, :, LH + PAD:LH + 2 * PAD], 0.0)
        ach = [sm.tile([P, HS, LH], bf16, name=f"acc{i}_{j}", tag="acch",
                       bufs=4) for j in range(NLH)]
        hh = [sm.tile([P, HS, LH], bf16, name=f"hhl{i}_{j}", tag="acch",
                      bufs=4) for j in range(NLH)]

        def conv_apply_transpose(lo):
            nc.scalar.activation(
                out=z_lh[:, lo], in_=x_sb[:, lo], func=FT.Identity,
                scale=rstd[:, lo:lo + 1], bias=bm[:, lo:lo + 1])
            ztmp = sm.tile([P, HS, P], bf16, name=f"ztmp{i}_{lo}", tag="ztmp",
                           bufs=4)
            nc.sync.dma_start_transpose(ztmp, z_lh[:, lo, :])
            j, c = lo // 4, (lo % 4) * P
            nc.vector.tensor_copy(out=zh[j][:, :, PAD + c:PAD + c + P],
                                  in_=ztmp)
            if lo == 4:  # halo: first PAD cols of lh1 -> right edge of lh0
                nc.vector.tensor_copy(out=zh[0][:, :, PAD + LH:],
                                      in_=ztmp[:, :, 0:PAD])
            if lo == 3:  # halo: last PAD cols of lh0 -> left edge of lh1
                nc.vector.tensor_copy(out=zh[1][:, :, 0:PAD],
                                      in_=ztmp[:, :, P - PAD:P])

        def conv_dw(lh):
            # depthwise on PE: psum[c',l] += D_k[c,c']z[c,l+k], D_k=diag(w_k*g)
            # split into two 64x64 diagonal quadrants packed at (0,0)/(64,64)
            for s in range(HS):
                ps = psum_sc(f"dwps{i}_{s}_{lh}")
                for k in range(KSZ):
                    nc.tensor.matmul(
                        ps, dmat[:, s, k, :], zh[lh][:, s, k:k + LH],
                        start=(k == 0), stop=(k == KSZ - 1))
                nc.scalar.activation(
                    out=ach[lh][:, s], in_=ps,
                    func=FT.Identity, bias=bprime[:, s:s + 1], scale=1.0)

        def conv_pw(lh):
            for ot in range(HS):
                ps = psum_sc(f"pwps{i}_{ot}_{lh}")
                for ks in range(HS):
                    nc.tensor.matmul(
                        ps, pwt_sb[:, ks, ts(ot, P)], ach[lh][:, ks],
                        start=(ks == 0), stop=(ks == HS - 1))
                nc.scalar.activation(
                    out=hh[lh][:, ot], in_=ps,
                    func=FT.Relu, bias=pwb_sb[:, i, ot:ot + 1], scale=1.0)

        def conv_tail(lh):
            ht = sm.tile([P, HS, 4, P], bf16, name=f"ht{i}_{lh}", tag="acch",
                         bufs=4)
            nc.sync.dma_start_transpose(ht, hh[lh][:, :, :])
            for lo in range(4 * lh, 4 * lh + 4):
                xv = x_sb[:, lo].rearrange("p (a b) -> p a b", b=P)
                eng = nc.gpsimd if lo % 3 == 2 else nc.vector
                eng.tensor_add(xv, xv, ht[:, :, lo % 4, :])

        # pipeline the two l-halves
        for lo in range(5):
            conv_apply_transpose(lo)
        conv_dw(0)
        for lo in range(5, LT):
            conv_apply_transpose(lo)
        conv_pw(0)
        conv_dw(1)
        conv_tail(0)
        conv_pw(1)
        conv_tail(1)

    if PHASES == "conv":
        nc.sync.dma_start(
            out=out_d[:, :].rearrange("(lo p) h -> p lo h", p=P), in_=x_sb)
        return

    # ---------------- attention ----------------
    z_lh = layer_norm("a")
    zq_t = transpose_lh_to_int(z_lh, "a", 0)  # [P(h), LT, HS, P(l)]

    # per-headpair q/k/v tiles, emitted PAIR-major so scores(hp0) starts
    # after 3 projection groups instead of 9; vt2 per-pair so AV(hp) only
    # waits its own pair's V rearrangement
    q_t, k_t, v_t = ({}, {}, {})
    vt_r, vt2 = ({}, {})
    w_sbs = {}
    for wname, w_d in (("q", wq_d), ("k", wk_d), ("v", wv_d)):
        w_sb = w8.tile([P, HS, H], bf16, name=f"w{wname}sb", tag="w8")
        nc.sync.dma_start(out=w_sb, in_=w_d[:, :].rearrange("(s p) o -> p s o", p=P))
        w_sbs[wname] = w_sb
    for ot in range(HS):
        for wname, store in (("q", q_t), ("k", k_t), ("v", v_t)):
            prj = sm.tile([P, L], bf16, name=f"{wname}sb{ot}", tag="qkv",
                          bufs=18)
            store[ot] = prj
            for lh in range(NLH):
                ps = psum_sc(f"qkvps{wname}{ot}_{lh}")
                for ks in range(HS):
                    nc.tensor.matmul(
                        ps, w_sbs[wname][:, ks, ts(ot, P)],
                        zq_t[lh][:, :, ks, :],
                        start=(ks == 0), stop=(ks == HS - 1))
                dst = prj[:, lh * LH:(lh + 1) * LH]
                if wname == "v":
                    nc.scalar.copy(out=dst, in_=ps)
                else:
                    nc.vector.tensor_copy(out=dst, in_=ps)
        # V^T per pair: [dv(2 heads), m] -> [m, dv], plus ones column
        vr = sm.tile([P, LT, P], bf16, name=f"vtr{ot}", tag="qkv", bufs=18)
        nc.sync.dma_start_transpose(vr, v_t[ot][:, :])
        vt_r[ot] = vr
        v2 = sm.tile([P, LT, 2, DK + 1], bf16, name=f"vt2_{ot}", tag="qkv",
                     bufs=18)
        for mo in range(LT):
            nc.vector.tensor_copy(out=v2[:, mo, 0, 0:DK],
                                  in_=vr[:, mo, 0:DK])
            nc.vector.tensor_copy(out=v2[:, mo, 1, 0:DK],
                                  in_=vr[:, mo, DK:2 * DK])
        nc.vector.memset(v2[:, :, :, DK:DK + 1], 1.0)
        vt2[ot] = v2

    oT_sb = w8.tile([P, HS, L], bf16, name="oT_sb", tag="w8")
    ones1 = pp.tile([P, DK], bf16, name="ones1")
    nc.vector.memset(ones1, 1.0)
    sc_scale = 1.0 / float(np.sqrt(DK))
    for hp in range(HS):
        hA, hB = 2 * hp, 2 * hp + 1
        pA = [p16.tile([P, LT, LH], bf16, name=f"pA{hp}_{j}", tag="p16",
                       bufs=4) for j in range(NLH)]
        pB = [p16.tile([P, LT, LH], bf16, name=f"pB{hp}_{j}", tag="p16",
                       bufs=4) for j in range(NLH)]
        for lh in range(NLH):
            for mo in range(LT):
                psA = psum_sc(f"sA{hp}_{mo}_{lh}")
                psB = psum_sc(f"sB{hp}_{mo}_{lh}")
                nc.tensor.matmul(
                    psA, k_t[hp][0:DK, ts(mo, P)],
                    q_t[hp][0:DK, lh * LH:(lh + 1) * LH],
                    start=True, stop=True, tile_position=(0, 0))
                nc.tensor.matmul(
                    psB, k_t[hp][DK:P, ts(mo, P)],
                    q_t[hp][DK:P, lh * LH:(lh + 1) * LH],
                    start=True, stop=True, tile_position=(DK, 0))
                nc.scalar.activation(
                    out=pA[lh][:, mo, :], in_=psA, func=FT.Exp,
                    bias=maskb[:, mo:mo + 1], scale=sc_scale)
                nc.scalar.activation(
                    out=pB[lh][:, mo, :], in_=psB, func=FT.Exp,
                    bias=maskb[:, mo:mo + 1], scale=sc_scale)

        for (hh, ph, part0) in ((hA, pA, True), (hB, pB, False)):
            rb = sm.tile([P, L], bf16, name=f"rb{hh}", tag="rbc", bufs=2)
            if not part0:
                otmp = sm.tile([DK, L], bf16, name=f"ot{hh}", tag="otmp", bufs=2)
            for lhx in range(NLH):
                lsl = slice(lhx * LH, (lhx + 1) * LH)
                rtmp = sm.tile([P, LH], bf16, name=f"rt{hh}_{lhx}", tag="rtmp",
                               bufs=2)
                r0 = sm.tile([P, LH], bf16, name=f"r0{hh}_{lhx}", tag="r0",
                             bufs=2)
                pso = psp.tile([DK + 1, LH], f32, name=f"av{hh}_{lhx}",
                               tag="av", bufs=3)
                for mo in range(LT):
                    nc.tensor.matmul(pso, vt2[hp][:, mo, hh % 2, 0:DK + 1],
                                     ph[lhx][:, mo, :],
                                     start=(mo == 0), stop=(mo == LT - 1))
                # reciprocal of rowsum (lives at partition DK=64)
                with nc.allow_low_precision(reason="softmax denom in bf16"):
                    nc.vector.reciprocal(out=rtmp[DK:DK + 1, :],
                                         in_=pso[DK:DK + 1, :])
                nc.sync.dma_start(out=r0[0:1, :], in_=rtmp[DK:DK + 1, :])
                # broadcast across DK partitions: ones [1,DK] outer r0 [1,LH]
                psR = psp.tile([P, LH], f32, name=f"psR{hh}_{lhx}",
                               tag="av", bufs=3)
                nc.tensor.matmul(psR[0:DK, :], ones1[0:1, :], r0[0:1, :],
                                 start=True, stop=True)
                nc.vector.tensor_copy(out=rb[0:DK, lsl], in_=psR[0:DK, :])
                if part0:
                    nc.vector.scalar_tensor_tensor(
                        out=oT_sb[0:DK, hp, lsl], in0=pso[0:DK, :], scalar=0.0,
                        in1=rb[0:DK, lsl], op0=OP.bypass, op1=OP.mult)
                else:
                    nc.vector.scalar_tensor_tensor(
                        out=otmp[:, lsl], in0=pso[0:DK, :], scalar=0.0,
                        in1=rb[0:DK, lsl], op0=OP.bypass, op1=OP.mult)
            if not part0:
                nc.sync.dma_start(out=oT_sb[DK:P, hp, :], in_=otmp)

    # output projection + residual (per l-half tails)
    pjt_sb = w8.tile([P, HS, H], bf16, name="pjt_sb", tag="w8")
    nc.sync.dma_start(out=pjt_sb, in_=pjt_d[:, :].rearrange("(s p) o -> p s o", p=P))
    for lh in range(NLH):
        pr_h = sm.tile([P, HS, LH], bf16, name=f"prh{lh}", tag="acch", bufs=4)
        for ot in range(HS):
            ps = psum_sc(f"prps{ot}_{lh}")
            for ds in range(HS):
                nc.tensor.matmul(ps, pjt_sb[:, ds, ts(ot, P)],
                                 oT_sb[:, ds, lh * LH:(lh + 1) * LH],
                                 start=(ds == 0), stop=(ds == HS - 1))
            nc.scalar.activation(
                out=pr_h[:, ot], in_=ps,
                func=FT.Identity, bias=pjb_sb[:, ot:ot + 1], scale=1.0)
        prt = sm.tile([P, HS, 4, P], bf16, name=f"prt{lh}", tag="acch", bufs=4)
        nc.sync.dma_start_transpose(prt, pr_h[:, :, :])
        for lo in range(4 * lh, 4 * lh + 4):
            xv = x_sb[:, lo].rearrange("p (a b) -> p a b", b=P)
            eng = nc.gpsimd if lo % 3 == 2 else nc.vector
            eng.tensor_add(xv, xv, prt[:, :, lo % 4, :])

    if PHASES == "attn":
        nc.sync.dma_start(
            out=out_d[:, :].rearrange("(lo p) h -> p lo h", p=P), in_=x_sb)
        return

    # ---------------- FFN ----------------
    z_lh = layer_norm("f")
    zf_t = transpose_lh_to_int(z_lh, "f", 1)

    # h1 in 4 quarter tiles so W2's accumulation starts after each quarter
    h1q = [w8.tile([P, FS // 4, L], bf16, name=f"h1q{j}", tag="w8")
           for j in range(4)]
    w2_sb = w8.tile([P, FS, H], bf16, name="w2_sb", tag="w2", bufs=1)
    nc.sync.dma_start(out=w2_sb, in_=w2t_d[:, :].rearrange("(s p) o -> p s o", p=P))
    w1h = [w8.tile([P, 2, FFN], bf16, name=f"w1h{j}", tag="w8")
           for j in range(2)]
    for j in range(2):
        nc.sync.dma_start(
            out=w1h[j],
            in_=w1t_d[:, :].rearrange("(s p) o -> p s o", p=P)[:, 2 * j:2 * j + 2])
    for ot in range(FS):
        htile = h1q[ot // (FS // 4)]
        for lh in range(NLH):
            ps = psum_sc(f"f1ps{ot}_{lh}")
            for ks in range(HS):
                nc.tensor.matmul(
                    ps, w1h[ks // 2][:, ks % 2, ts(ot, P)],
                    zf_t[lh][:, :, ks, :],
                    start=(ks == 0), stop=(ks == HS - 1))
            nc.scalar.activation(
                out=htile[:, ot % (FS // 4), lh * LH:(lh + 1) * LH], in_=ps,
                func=FT.Relu, bias=b1_sb[:, ot:ot + 1], scale=1.0)

    for lh in range(NLH):
        o2_h = sm.tile([P, HS, LH], bf16, name=f"o2h{lh}", tag="acch", bufs=4)
        for ot in range(HS):
            ps = psum_sc(f"f2ps{ot}_{lh}")
            for ks in range(FS):
                htile = h1q[ks // (FS // 4)]
                nc.tensor.matmul(ps, w2_sb[:, ks, ts(ot, P)],
                                 htile[:, ks % (FS // 4), lh * LH:(lh + 1) * LH],
                                 start=(ks == 0), stop=(ks == FS - 1))
            nc.scalar.activation(
                out=o2_h[:, ot], in_=ps,
                func=FT.Identity, bias=b2_sb[:, ot:ot + 1], scale=1.0)
        o2t = sm.tile([P, HS, 4, P], bf16, name=f"o2t{lh}", tag="acch", bufs=4)
        nc.sync.dma_start_transpose(o2t, o2_h[:, :, :])
        for lo in range(4 * lh, 4 * lh + 4):
            xv = x_sb[:, lo].rearrange("p (a b) -> p a b", b=P)
            eng = nc.gpsimd if lo % 3 == 2 else nc.vector
            eng.tensor_add(xv, xv, o2t[:, :, lo % 4, :])

    nc.sync.dma_start(
        out=out_d[:, :].rearrange("(lo p) h -> p lo h", p=P), in_=x_sb)


_NC_CACHE = {}
LAST_RESULTS = None


def get_nc():
    if "nc" not in _NC_CACHE:
        _NC_CACHE["nc"] = build_nc()
    return _NC_CACHE["nc"]


def prep_inputs(x, x_mask, pos_emb, cnn_gamma, cnn_beta, cnn_dw_w, cnn_dw_b,
                cnn_pw_w, cnn_pw_b, attn_gamma, attn_beta, w_qs, w_ks, w_vs,
                proj_w, proj_b, ffn_gamma, ffn_beta, ffn_w1, ffn_b1,
                ffn_w2, ffn_b2):
    bf = ml_dtypes.bfloat16
    f = np.float32
    C = np.ascontiguousarray

    x = np.asarray(x, f)
    mask = np.asarray(x_mask).astype(np.uint8)
    base = dict(
        pos=C(np.asarray(pos_emb, f)[0, :L, :]),
        cg=C(np.asarray(cnn_gamma, f)), cb=C(np.asarray(cnn_beta, f)),
        dww=C(np.asarray(cnn_dw_w, f)), dwb=C(np.asarray(cnn_dw_b, f)),
        pwt=C(np.transpose(np.asarray(cnn_pw_w, f), (0, 2, 1))).astype(bf),
        pwb=C(np.asarray(cnn_pw_b, f)),
        ag=C(np.asarray(attn_gamma, f)), ab=C(np.asarray(attn_beta, f)),
        wq=C(np.transpose(np.asarray(w_qs, f), (1, 0, 2)).reshape(H, H)).astype(bf),
        wk=C(np.transpose(np.asarray(w_ks, f), (1, 0, 2)).reshape(H, H)).astype(bf),
        wv=C(np.transpose(np.asarray(w_vs, f), (1, 0, 2)).reshape(H, H)).astype(bf),
        pjt=C(np.asarray(proj_w, f).T).astype(bf),
        pjb=C(np.asarray(proj_b, f)),
        fg=C(np.asarray(ffn_gamma, f)), fb=C(np.asarray(ffn_beta, f)),
        w1t=C(np.asarray(ffn_w1, f).T).astype(bf),
        b1=C(np.asarray(ffn_b1, f)),
        w2t=C(np.asarray(ffn_w2, f).T).astype(bf),
        b2=C(np.asarray(ffn_b2, f)),
    )
    in_maps = [dict(base, x=C(x[b]), mask=C(mask[b])) for b in range(B)]
    return in_maps


def kernel(**inputs):
    global LAST_RESULTS
    from concourse.bass_utils import run_bass_kernel_spmd
    nc = get_nc()
    in_maps = prep_inputs(**inputs)
    res = run_bass_kernel_spmd(nc, in_maps, list(range(B)))
    LAST_RESULTS = res
    return np.stack([r["out"] for r in res.results]).astype(np.float32)
